# revision 18
# baseline (speedup 1.0000x reference)
"""Bass/Trainium2 kernel for GQA attention (B=1, LQ=LK=2048, D=4096,
H=32, KVH=8, DH=128) distributed over 8 NeuronCores, tensor-parallel by
heads: core i owns kv-head i and its 4 query heads.

Per-core pipeline (all matmuls bf16, accumulation fp32 in PSUM):
  1. qT/kT/v projections from host-pretiled hidden-state slabs (each
     slab one fully-contiguous 1MB DMA).  dh^-0.5 is folded into Wq on
     the host.
  2. scoresT = kT . qT per 128k x 512q block (causal blocks only);
     ACT computes exp(score) straight out of PSUM; DVE multiplies by
     the host-precomputed exp(bias+mask) tile (bf16 2x mode) and
     accumulates the softmax denominator e_acc in SBUF.
  3. U_T += v . eT on PE; S = ones . e_acc (single matmul per unit);
     1/S as exp(-ln S) on ACT; out_head = U_T * (1/S) on DVE.
  4. partial_out = attnT . Wo_shard ; host sums the 8 partials.
"""
import os
import sys
import types

import numpy as np
import ml_dtypes

sys.path.insert(0, '/opt/trn_rl_repo')

BF16 = ml_dtypes.bfloat16

# ---------------------------------------------------------------- axon shim
def _install_axon_hooks():
    """Provide antenv.axon_hooks (absent in this image) so that
    run_bass_kernel_spmd(trace=True) / BASS_TRACE=1 can capture NTFF
    profiles instead of crashing on import."""
    if "antenv.axon_hooks" in sys.modules:
        return
    state = {"hook": None}
    mod = types.ModuleType("antenv.axon_hooks")
    mod.set_axon_ntff_profile_hook = lambda h: state.__setitem__("hook", h)
    mod.get_axon_ntff_profile_hook = lambda: state["hook"]
    sys.modules["antenv.axon_hooks"] = mod
    try:
        from trn_agent_boot.trn_boot import _ntff_profile_via_ctypes
        mod.set_axon_ntff_profile_hook(
            _ntff_profile_via_ctypes('/opt/axon/libaxon_pjrt.so'))
    except Exception:
        pass


_install_axon_hooks()

import concourse.bass as bass
import concourse.tile as tile
from concourse.tile import add_dep_helper
from concourse import mybir
from concourse.bass_utils import run_bass_kernel_spmd
from concourse.alu_op_type import AluOpType
from concourse.masks import make_identity

# ---------------------------------------------------------------- constants
B, LQ, LK = 1, 2048, 2048
D, H, KVH, DH = 4096, 32, 8, 128
G = H // KVH          # 4 query heads per kv head
N_CORES = 8
NH = H // N_CORES     # 4 heads per core
KO = D // 128         # 32 contraction chunks for the projections
QC = 512              # q free-dim chunk for attention blocks
NEG = -30000.0        # additive mask value (exp -> exactly 0 in fp32)

FP32 = mybir.dt.float32
DT = mybir.dt.bfloat16


def _split_drain_tile_context():
    """TileContext whose final drain splits its semaphore waits across
    multiple drain instructions — walrus in this container rejects CTRL
    instructions carrying more than one sync wait."""
    import bass_rust

    class SplitDrainTC(tile.TileContext):
        def _drain_and_barrier(self, tick_clock, wait_clock):
            drain_inst = self.nc.sync.drain()
            wait_clock.add_sem_waits(
                drain_inst.ins, tile.ScopedClock({None: tick_clock.global_clock})
            )
            si = drain_inst.ins.sync_info
            if si is not None and si.on_wait and len(si.on_wait) > 1:
                waits = list(si.on_wait)
                si.on_wait = waits[:1]
                drain_inst.ins.sync_info = si
                for w in waits[1:]:
                    d2 = self.nc.sync.drain()
                    d2.ins.sync_info = bass_rust.SyncInfo(on_wait=[w], on_update=[])

            self.nc.all_engine_barrier()
            assert self.sems is not None
            popped = self.nc._tile_sem_poison_stack.pop()
            assert popped is self._sem_poison
            self.nc.clear_and_free_semaphores(list(self.sems.allocated().values()))
            self.nc.all_engine_barrier()

    return SplitDrainTC


def _bias_offsets(nk_per_qc):
    """Column offsets of each (h, qc) chunk in the flat ebias tensor."""
    n_s = LQ // QC
    per_h = sum(nk_per_qc) * QC
    offs = {}
    for h in range(NH):
        acc = h * per_h
        for qc in range(n_s):
            offs[(qc, h)] = acc
            acc += nk_per_qc[qc] * QC
    return offs, NH * per_h


def build_graph(nk_per_qc, off_table):
    """Build the single-core SPMD graph. nk_per_qc[qc] = number of 128-wide
    key chunks to process for query chunk qc; off_table[qc][kc] = first live
    q column (multiple of 128) of block (kc, qc). Both derived from the
    attention mask on the host."""
    nc = bass.Bass("TRN2", target_bir_lowering=False, debug=False,
                   num_devices=N_CORES)

    n_s = LQ // QC        # 4 query chunks of 512
    n_m = LQ // 128       # 16 seq chunks of 128
    bias_off, bias_cols = _bias_offsets(nk_per_qc)

    # host-pretiled inputs: every DMA below is per-partition contiguous
    hq_t = nc.dram_tensor("hq_t", [4 * n_s, 128, KO // 4 * QC], DT,
                          kind="ExternalInput").ap()
    hkv_t = nc.dram_tensor("hkv_t", [4 * n_s, 128, KO // 4 * QC], DT,
                           kind="ExternalInput").ap()
    wq_t = nc.dram_tensor("wq_t", [128, KO, NH * DH], DT, kind="ExternalInput").ap()
    wk_t = nc.dram_tensor("wk_t", [128, KO, DH], DT, kind="ExternalInput").ap()
    wv_t = nc.dram_tensor("wv_t", [128, KO, DH], DT, kind="ExternalInput").ap()
    wo_t = nc.dram_tensor("wo_t", [128, NH, D], DT, kind="ExternalInput").ap()
    ebias_t = nc.dram_tensor("ebias_t", [128, bias_cols], DT,
                             kind="ExternalInput").ap()
    out = nc.dram_tensor("out", [LQ, D], DT, kind="ExternalOutput").ap()

    TC = _split_drain_tile_context()
    with TC(nc) as tc:
        with tc.tile_pool(name="weights", bufs=1) as wpool, \
             tc.tile_pool(name="persist", bufs=1) as ppool, \
             tc.tile_pool(name="bias", bufs=3) as bias_pool:
            ones_sb = wpool.tile([128, 128], DT)
            nc.vector.memset(ones_sb[:], 1.0)
            ident_sb = wpool.tile([128, 128], DT)
            # preload the exp table set while projections run
            warm_sb = wpool.tile([128, 1], FP32)
            nc.scalar.activation(out=warm_sb[:], in_=ones_sb[:, 0:1],
                                 func=mybir.ActivationFunctionType.Exp)

            bias_tiles = {}

            def fetch_bias(qc, h, not_before=None):
                # split into <=4-chunk pieces so the transfer spreads over
                # multiple DMA queues (single-queue bandwidth is the binding
                # constraint for multi-MB tiles)
                nk = nk_per_qc[qc]
                o = bias_off[(qc, h)]
                bias_sb = bias_pool.tile([128, nk, QC], DT,
                                         tag="bias", name=f"bias_{qc}_{h}")
                for a in range(0, nk, 4):
                    b = min(nk, a + 4)
                    dma = nc.gpsimd.dma_start(
                        out=bias_sb[:, a:b, :],
                        in_=ebias_t[:, o + a * QC:o + b * QC].rearrange(
                            "p (ko q) -> p ko q", q=QC))
                    if not_before is not None:
                        add_dep_helper(dma.ins, not_before,
                                       reason="keep bias off startup HBM")
                bias_tiles[(qc, h)] = bias_sb

            # persistent activations
            wo_sb = ppool.tile([128, NH, D], DT)      # [hd_in, h, d_out]
            qT_sb = ppool.tile([128, NH, LQ], DT)     # [dh, h, q]
            kT_sb = ppool.tile([128, LK], DT)         # [dh, k]
            v_sb = ppool.tile([128, LK // 128, DH], DT)   # [k_in, k_blk, dh]
            un_sb = ppool.tile([128, NH, LQ], DT)     # normalized U_T

            # ---------------- stage 1: projections ----------------
            with tc.tile_pool(name="w1", bufs=1) as w1pool, \
                 tc.tile_pool(name="slab", bufs=8) as slab_pool, \
                 tc.tile_pool(name="proj_ps", bufs=3, space="PSUM") as proj_ps, \
                 tc.tile_pool(name="vtr_ps", bufs=2, space="PSUM") as vtr_ps, \
                 nc.named_scope("proj"):
                slab_tiles = {}

                def fetch_slab(kind, s, quarter, pieces=(4, 4), eng=None):
                    # each slab is one contiguous [128, 4096] DRAM block,
                    # transferred as independent piece-tiles so matmuls can
                    # start on piece 0 while later pieces are in flight and
                    # the transfer spreads over parallel DMA queues
                    src = hkv_t if kind == "kv" else hq_t
                    parts = []
                    last = None
                    a = 0
                    for w in pieces:
                        b = a + w
                        part = slab_pool.tile(
                            [128, w, QC], DT, tag="slab",
                            name=f"slab_{kind}_{s}_{quarter}_{a}")
                        last = (eng or nc.gpsimd).dma_start(
                            out=part[:],
                            in_=src[s * 4 + quarter][:, a * QC:b * QC].rearrange(
                                "p (ko q) -> p ko q", q=QC))
                        parts.append((part, a, b))
                        a = b
                    slab_tiles[(kind, s, quarter)] = parts
                    return last

                def slab_chunk(parts, kl):
                    for part, a, b in parts:
                        if a <= kl < b:
                            return part[:, kl - a, :]
                    raise KeyError(kl)

                # the whole first kv step is prefetched in small pieces,
                # split across the gpsimd SWDGE and the ACT HWDGE rings, so
                # the PE can start on piece 0 while the rest streams in
                fetch_slab("kv", 0, 0, pieces=(2, 2, 2, 2))
                fetch_slab("kv", 0, 1, pieces=(4, 4), eng=nc.scalar)
                wk_sb = w1pool.tile([128, KO, DH], DT)
                nc.sync.dma_start(out=wk_sb[:, 0:2, :], in_=wk_t[:, 0:2, :])
                nc.sync.dma_start(out=wk_sb[:, 2:8, :], in_=wk_t[:, 2:8, :])
                nc.sync.dma_start(out=wk_sb[:, 8:, :], in_=wk_t[:, 8:, :])
                wv_sb = w1pool.tile([128, KO, DH], DT)
                nc.sync.dma_start(out=wv_sb[:, 0:16, :], in_=wv_t[:, 0:16, :])
                nc.sync.dma_start(out=wv_sb[:, 16:, :], in_=wv_t[:, 16:, :])
                vT_sb = w1pool.tile([128, LK], DT)        # [dh, k]
                wq_sb = w1pool.tile([128, KO, NH * DH], DT)
                wq_dmas = []
                for g in range(4):
                    wq_dmas.append(nc.sync.dma_start(
                        out=wq_sb[:, g * 8:(g + 1) * 8, :],
                        in_=wq_t[:, g * 8:(g + 1) * 8, :]))
                early_markers = {}

                def kv_step(s):
                    kt_ps = proj_ps.tile([128, QC], FP32, tag="pps",
                                         name=f"ktps_{s}")
                    vt_ps = proj_ps.tile([128, QC], FP32, tag="pps2",
                                         name=f"vtps_{s}")
                    for quarter in range(4):
                        if (("kv", s, quarter)) not in slab_tiles:
                            dma = fetch_slab("kv", s, quarter)
                            if s == 0 and quarter == 2:
                                early_markers['kv0q2'] = dma.ins
                        parts = slab_tiles.pop(("kv", s, quarter))
                        for kl in range(KO // 4):
                            ko = quarter * (KO // 4) + kl
                            nc.tensor.matmul(kt_ps[:], lhsT=wk_sb[:, ko, :],
                                             rhs=slab_chunk(parts, kl),
                                             start=(ko == 0), stop=(ko == KO - 1))
                        for kl in range(KO // 4):
                            ko = quarter * (KO // 4) + kl
                            nc.tensor.matmul(vt_ps[:], lhsT=wv_sb[:, ko, :],
                                             rhs=slab_chunk(parts, kl),
                                             start=(ko == 0), stop=(ko == KO - 1))
                    nc.scalar.copy(out=kT_sb[:, s * QC:(s + 1) * QC], in_=kt_ps[:])
                    return nc.scalar.copy(out=vT_sb[:, s * QC:(s + 1) * QC],
                                          in_=vt_ps[:])

                def q_step(s):
                    q_pss = [proj_ps.tile([128, QC], FP32,
                                          tag=("pps" if h % 2 == 0 else "pps2"),
                                          name=f"qps_{s}_{h}")
                             for h in range(NH)]
                    for quarter in range(4):
                        fetch_slab("q", s, quarter)
                        parts = slab_tiles.pop(("q", s, quarter))
                        for h in range(NH):
                            for kl in range(KO // 4):
                                ko = quarter * (KO // 4) + kl
                                nc.tensor.matmul(
                                    q_pss[h][:],
                                    lhsT=wq_sb[:, ko, h * DH:(h + 1) * DH],
                                    rhs=slab_chunk(parts, kl),
                                    start=(ko == 0), stop=(ko == KO - 1))
                    for h in range(NH):
                        nc.scalar.copy(out=qT_sb[:, h, s * QC:(s + 1) * QC],
                                       in_=q_pss[h][:])

                def v_transpose():
                    make_identity(nc, ident_sb[:])
                    for blk in range(LK // 128):
                        tp = vtr_ps.tile([128, 128], DT, tag="vtr",
                                         name=f"vtr_{blk}")
                        nc.tensor.transpose(
                            tp[:], vT_sb[:, blk * 128:(blk + 1) * 128],
                            ident_sb[:])
                        nc.scalar.copy(out=v_sb[:, blk, :], in_=tp[:])

                kv_step(0)
                for dma in wq_dmas:
                    add_dep_helper(dma.ins, early_markers['kv0q2'],
                                   reason="keep wq off the startup HBM burst")
                q_step(0)
                kv_step(1)
                q_step(1)
                marker = kv_step(2).ins
                q_step(2)
                kv_step(3)
                v_transpose()
                # wo/bias are not needed until the attention phase; the
                # explicit dep keeps the scheduler from hoisting their
                # transfers into the HBM-critical startup window
                for hh in range(NH):
                    dma = nc.sync.dma_start(out=wo_sb[:, hh, :],
                                            in_=wo_t[:, hh, :])
                    add_dep_helper(dma.ins, marker,
                                   reason="keep wo off startup HBM")
                for h in range(NH):
                    fetch_bias(0, h, not_before=marker)
                q_step(3)

            # ------- stage 2+3: attention interleaved with out-proj -------
            with tc.tile_pool(name="att_sb", bufs=4) as att_sb, \
                 tc.tile_pool(name="eacc_sb", bufs=2) as eacc_pool, \
                 tc.tile_pool(name="sc_ps", bufs=4, space="PSUM") as sc_pool, \
                 tc.tile_pool(name="acc_ps", bufs=2, space="PSUM") as acc_pool, \
                 tc.tile_pool(name="osb", bufs=3) as out_pool, \
                 tc.tile_pool(name="ops", bufs=2, space="PSUM") as out_ps, \
                 nc.named_scope("attn_wo"):

                pending = {"tail": None}

                def emit_attn_unit(qc, h, filler=None):
                    nk = nk_per_qc[qc]
                    if (qc, h) not in bias_tiles:
                        fetch_bias(qc, h)
                    bias_sb = bias_tiles.pop((qc, h))
                    # prefetch the next unit's bias with a full unit of lead
                    nxt = (qc, h + 1) if h + 1 < NH else (qc + 1, 0)
                    if nxt[0] < n_s and nxt not in bias_tiles:
                        fetch_bias(*nxt)
                    u_ps = acc_pool.tile([128, QC], FP32, tag="ups",
                                         name=f"ups_{qc}_{h}")
                    e_acc = eacc_pool.tile([128, QC], DT, tag="eacc",
                                           name=f"eacc_{qc}_{h}")
                    e_tiles = {}

                    def off_of(kc):
                        # columns left of the first live one are fully
                        # masked for this key block — skip them
                        return off_table[qc][kc]

                    # software pipeline: score/exp/mult for kc emitted two
                    # iterations before the accumulation matmul of kc-2, so
                    # the exp->mult chain never stalls the PE
                    lead = min(2, nk - 1)
                    for kc in range(nk + lead):
                        if kc < nk:
                            off = off_of(kc)
                            sc_ps = sc_pool.tile([128, QC], FP32, tag="scps",
                                                 name=f"scps_{qc}_{h}_{kc}")
                            nc.tensor.matmul(
                                sc_ps[:, off:],
                                lhsT=kT_sb[:, kc * 128:(kc + 1) * 128],
                                rhs=qT_sb[:, h, qc * QC + off:(qc + 1) * QC],
                                start=True, stop=True)
                            er_sb = att_sb.tile([128, QC], DT, tag="eraw",
                                                name=f"eraw_{qc}_{h}_{kc}")
                            nc.scalar.activation(
                                out=er_sb[:, off:], in_=sc_ps[:, off:],
                                func=mybir.ActivationFunctionType.Exp)
                            e_sb = att_sb.tile([128, QC], DT, tag="esb",
                                               name=f"esb_{qc}_{h}_{kc}")
                            nc.vector.tensor_mul(
                                e_sb[:, off:], er_sb[:, off:],
                                bias_sb[:, kc, off:])
                            e_tiles[kc] = e_sb
                            # e_acc accumulates the softmax denominator; the
                            # kc=1 add also materializes it (no aliasing with
                            # the e tiles the AV matmuls read)
                            if kc == 1:
                                if off > 0:
                                    nc.vector.tensor_copy(
                                        out=e_acc[:, :off],
                                        in_=e_tiles[0][:, :off])
                                nc.vector.tensor_add(
                                    e_acc[:, off:], e_tiles[0][:, off:],
                                    e_sb[:, off:])
                            elif kc >= 2:
                                nc.vector.tensor_add(
                                    e_acc[:, off:], e_acc[:, off:],
                                    e_sb[:, off:])
                        if kc == lead:
                            # the previous unit's normalization tail and an
                            # out-projection chunk run here, while ACT/DVE
                            # produce this unit's first e tiles — keeps the
                            # PE busy across the unit boundary
                            if pending["tail"] is not None:
                                pending["tail"]()
                                pending["tail"] = None
                            if filler is not None:
                                filler()
                        if kc >= lead:
                            off = off_of(kc - lead)
                            e_prev = e_tiles.pop(kc - lead)
                            nc.tensor.matmul(u_ps[:, off:],
                                             lhsT=v_sb[:, kc - lead, :],
                                             rhs=e_prev[:, off:],
                                             start=(kc == lead),
                                             stop=(kc == nk + lead - 1),
                                             skip_group_check=True)

                    def tail():
                        # softmax denominator: one matmul over the DVE-summed
                        # e_acc, then 1/S as exp(-ln S) on ACT (Ln and Exp
                        # share the natural_log_exp table set); the final
                        # normalization multiply reads U straight from PSUM.
                        # For the last head of a query chunk the whole chain
                        # runs in halves: the first out-projection row block
                        # only waits on the first half.
                        s_bc = sc_pool.tile([128, QC], FP32, tag="scps",
                                            name=f"sps_{qc}_{h}")
                        lns_sb = att_sb.tile([128, QC], FP32, tag="lns",
                                             name=f"lns_{qc}_{h}")
                        sinv_sb = att_sb.tile([128, QC], DT, tag="sinv",
                                              name=f"sinv_{qc}_{h}")
                        halves = ([(0, QC // 2), (QC // 2, QC)]
                                  if h == NH - 1 else [(0, QC)])
                        for c0, c1 in halves:
                            nc.tensor.matmul(s_bc[:, c0:c1], lhsT=ones_sb[:],
                                             rhs=e_acc[:, c0:c1],
                                             start=True, stop=True,
                                             skip_group_check=True)
                            nc.scalar.activation(
                                out=lns_sb[:, c0:c1], in_=s_bc[:, c0:c1],
                                func=mybir.ActivationFunctionType.Ln)
                            nc.scalar.activation(
                                out=sinv_sb[:, c0:c1], in_=lns_sb[:, c0:c1],
                                func=mybir.ActivationFunctionType.Exp,
                                scale=-1.0)
                            qsl = slice(qc * QC + c0, qc * QC + c1)
                            nc.vector.tensor_mul(un_sb[:, h, qsl],
                                                 u_ps[:, c0:c1],
                                                 sinv_sb[:, c0:c1])
                        if h == NH - 1:
                            wo_queue.extend(range(4 * qc, 4 * qc + 4))

                    pending["tail"] = tail

                def emit_wo_unit(m, split_dma=False):
                    o_sb = out_pool.tile([128, D], DT, tag="osb",
                                         name=f"osb_{m}")
                    for n in range(D // QC):
                        o_ps = out_ps.tile([128, QC], FP32, tag="ops",
                                           name=f"ops_{m}_{n}")
                        for h in range(NH):
                            nc.tensor.matmul(
                                o_ps[:],
                                lhsT=un_sb[:, h, m * 128:(m + 1) * 128],
                                rhs=wo_sb[:, h, n * QC:(n + 1) * QC],
                                start=(h == 0), stop=(h == NH - 1))
                        # alternate the PSUM->SBUF drain between DVE and ACT
                        # so neither engine becomes the wo-phase bottleneck
                        if n % 2 == 0:
                            nc.vector.tensor_copy(
                                out=o_sb[:, n * QC:(n + 1) * QC], in_=o_ps[:])
                        else:
                            nc.scalar.copy(
                                out=o_sb[:, n * QC:(n + 1) * QC], in_=o_ps[:])
                        # stream the row block out as soon as each half (or
                        # quarter, for the final unit) of it is drained
                        step = 2 if split_dma else 4
                        if n % step == step - 1:
                            c0, c1 = (n + 1 - step) * QC, (n + 1) * QC
                            nc.scalar.dma_start(
                                out=out[m * 128:(m + 1) * 128, c0:c1],
                                in_=o_sb[:, c0:c1])

                # interleave: wo chunks for qc are woven in as fillers as
                # soon as all of qc's units are normalized (the tail of unit
                # (qc, 3) — which runs inside unit (qc+1, 0) — extends the
                # queue)
                wo_queue = []
                units = [(qc, h) for qc in range(n_s) for h in range(NH)]
                for i, (qc, h) in enumerate(units):
                    def filler():
                        if wo_queue:
                            emit_wo_unit(wo_queue.pop(0))
                    emit_attn_unit(qc, h, filler=filler if i >= 1 else None)
                if pending["tail"] is not None:
                    pending["tail"]()
                    pending["tail"] = None
                while len(wo_queue) > 1:
                    emit_wo_unit(wo_queue.pop(0))
                emit_wo_unit(wo_queue.pop(0), split_dma=True)
    _split_waits(nc)
    return nc


def _split_waits(nc):
    """Walrus in this container accepts at most one sync wait per
    instruction: hoist extra waits onto same-engine nops placed directly
    before the instruction (identical semantics — the engine stream
    blocks on each in order)."""
    import bass_rust
    ctr = 0
    for f in nc.m.functions:
        for bb in f.blocks:
            new = []
            for inst in bb.instructions:
                si = inst.sync_info
                if si is not None and si.on_wait and len(si.on_wait) > 1:
                    waits = list(si.on_wait)
                    for w in waits[:-1]:
                        nop = bass_rust.InstNoOp(name=f"waitnop-{ctr}",
                                                 engine=inst.engine)
                        ctr += 1
                        nop.sync_info = bass_rust.SyncInfo(on_wait=[w],
                                                           on_update=[])
                        new.append(nop)
                    si.on_wait = waits[-1:]
                    inst.sync_info = si
                new.append(inst)
            bb.instructions = new


_CACHE = {}


def kernel(hidden_q, hidden_kv, attention_mask, position_bias, Wq, Wk, Wv, Wo):
    hq = np.asarray(hidden_q, dtype=np.float32)[0]      # [2048, 4096]
    hkv = np.asarray(hidden_kv, dtype=np.float32)[0]
    mask = np.asarray(attention_mask)[0]                # [2048, 2048] bool
    pb = np.asarray(position_bias, dtype=np.float32)    # [32, 2048, 2048]
    Wq = np.asarray(Wq, dtype=np.float32) * np.float32(DH ** -0.5)
    Wk = np.asarray(Wk, dtype=np.float32)
    Wv = np.asarray(Wv, dtype=np.float32)
    Wo = np.asarray(Wo, dtype=np.float32)

    # additive mask, transposed to [k, q]
    negT = np.where(mask, np.float32(0.0), np.float32(NEG)).T

    # which 128-key chunks are live for each 512-query chunk
    n_s = LQ // QC
    nk_per_qc = []
    off_table = []
    for qc in range(n_s):
        cols = negT[:, qc * QC:(qc + 1) * QC]            # [2048k, 512q]
        live = 0
        offs = []
        for kc in range(LK // 128):
            blk_live = np.any(cols[kc * 128:(kc + 1) * 128] != np.float32(NEG),
                              axis=0)                    # per q column
            if blk_live.any() and kc > 0:
                live = kc + 1
                offs.append(int(np.argmax(blk_live)) // 128 * 128)
            else:
                # kc == 0 must write the full psum width (accumulator init)
                live = max(live, kc + 1) if blk_live.any() else live
                offs.append(0)
        nk_per_qc.append(live)
        off_table.append(offs)
    key = (tuple(nk_per_qc), tuple(tuple(o) for o in off_table))

    if key not in _CACHE:
        _CACHE[key] = build_graph(nk_per_qc, off_table)
    nc = _CACHE[key]

    # slab-tiled hidden states: slab (s, quarter) is one contiguous
    # [128, 4096] block; element [p, ko*512+c] = X.T[quarter*1024+ko*128+p,
    # s*512+c]
    def tile_hidden(x):
        a = np.ascontiguousarray(x.T).astype(BF16)       # [4096, 2048]
        a = a.reshape(4, KO // 4, 128, n_s, QC)          # [qtr, ko, p, s, c]
        return np.ascontiguousarray(
            a.transpose(3, 0, 2, 1, 4).reshape(4 * n_s, 128, KO // 4 * QC))

    hq_t = tile_hidden(hq)
    hkv_t = tile_hidden(hkv)

    def tile_w_in(w):                                    # [(ko p), d] -> [p, ko, d]
        d = w.shape[1]
        return np.ascontiguousarray(
            w.reshape(KO, 128, d).transpose(1, 0, 2)).astype(BF16)

    in_maps = []
    for i in range(N_CORES):
        # exp(bias + mask) per head, causal-trimmed, flat [128, cols]
        chunks = []
        for h in range(NH):
            eb = np.exp(pb[NH * i + h].T + negT)         # [k, q] fp32
            for qc in range(n_s):
                nk = nk_per_qc[qc]
                c = eb[:nk * 128, qc * QC:(qc + 1) * QC]
                chunks.append(c.reshape(nk, 128, QC).transpose(1, 0, 2)
                               .reshape(128, nk * QC))
        ebias = np.concatenate(chunks, axis=1).astype(BF16)

        wo_i = Wo[i * NH * DH:(i + 1) * NH * DH, :]
        in_maps.append({
            "hq_t": hq_t,
            "hkv_t": hkv_t,
            "wq_t": tile_w_in(Wq[:, i * NH * DH:(i + 1) * NH * DH]),
            "wk_t": tile_w_in(Wk[:, i * DH:(i + 1) * DH]),
            "wv_t": tile_w_in(Wv[:, i * DH:(i + 1) * DH]),
            "wo_t": np.ascontiguousarray(
                wo_i.reshape(NH, 128, D).transpose(1, 0, 2)).astype(BF16),
            "ebias_t": ebias,
        })

    res = run_bass_kernel_spmd(nc, in_maps, list(range(N_CORES)))
    kernel.last_results = res

    acc = np.zeros((LQ, D), dtype=np.float32)
    for i in range(N_CORES):
        acc += res.results[i]["out"].astype(np.float32)
    return acc[None]
